# revision 1
# baseline (speedup 1.0000x reference)
"""AggrGATConv Trainium2 kernel: 8-core edge-parallel GAT with dst-sorted
window aggregation.

Design:
  inv-1 (device, node-sharded): h = feat @ W, el = h.Al, er = h.Ar per node.
  host: index-only edge prep (argsort by dst, 128-node windows, per-window
        padding to full 128-edge tiles) + row gathers of device-computed
        tables (pure data movement, no float arithmetic).
  inv-2 (device, edge-sharded): per window accumulate in PSUM
        [U | s] = sum_tiles B_t.T @ [expE*h | expE], then
        out = mean_heads(U / s) + mean(bias). Softmax uses exp without
        max-subtraction (logits are O(1) for this operator's scale).
"""
import math
import sys
import types
import contextlib
import ctypes

import numpy as np

import concourse.bacc as bacc
import concourse.tile as tile
import concourse.mybir as mybir
from concourse.bass import IndirectOffsetOnAxis  # noqa: F401
from concourse.bass_utils import run_bass_kernel_spmd

# ---------------- constants (hardcoded per problem spec) ----------------
N = 100000
E = 1600000
IN = 128
H, D = 4, 32
HD = H * D  # 128
NEG = 0.2
NCORES = 8
P = 128
WIN_PER_CORE = 98            # 98*128 = 12544 nodes per core
N_PAD = NCORES * WIN_PER_CORE * P  # 100352
NODES_PER_CORE = WIN_PER_CORE * P  # 12544

f32 = mybir.dt.float32
i32 = mybir.dt.int32


def _install_ntff_shim():
    """antenv.axon_hooks is absent in this image; provide the ctypes hook so
    trace=True works (used by test harness; harmless otherwise)."""
    if "antenv.axon_hooks" in sys.modules:
        return
    try:
        lib = ctypes.CDLL("/opt/axon/libaxon_pjrt.so")
        if not hasattr(lib, "axon_start_nrt_profile"):
            raise OSError("no symbol")
        lib.axon_start_nrt_profile.argtypes = [
            ctypes.POINTER(ctypes.c_int64), ctypes.c_size_t]
        lib.axon_start_nrt_profile.restype = ctypes.c_int64
        lib.axon_stop_nrt_profile.argtypes = [ctypes.c_char_p]
        lib.axon_stop_nrt_profile.restype = ctypes.c_int64

        @contextlib.contextmanager
        def _hook(output_dir, device_ids):
            import jax
            jax.devices()
            if device_ids:
                ids = (ctypes.c_int64 * len(device_ids))(*device_ids)
                rc = lib.axon_start_nrt_profile(ids, len(device_ids))
            else:
                rc = lib.axon_start_nrt_profile(None, 0)
            if rc != 0:
                raise RuntimeError(f"axon_start_nrt_profile rc={rc}")
            try:
                yield
            finally:
                n = lib.axon_stop_nrt_profile(str(output_dir).encode())
                print(f"profile: {n} file(s) -> {output_dir}", file=sys.stderr)

        hook = _hook
    except OSError:
        hook = None
    mod = types.ModuleType("antenv.axon_hooks")
    mod.get_axon_ntff_profile_hook = lambda: hook
    mod.set_axon_ntff_profile_hook = lambda h: None
    sys.modules["antenv.axon_hooks"] = mod


_install_ntff_shim()


# ---------------- invocation 1: node tables ----------------
def _build_inv1():
    nc = bacc.Bacc("TRN2", target_bir_lowering=False, debug=False,
                   num_devices=NCORES)
    featT = nc.declare_dram_parameter("featT", [P, NODES_PER_CORE], f32,
                                      isOutput=False)
    W_in = nc.declare_dram_parameter("W", [IN, HD], f32, isOutput=False)
    WT_in = nc.declare_dram_parameter("WT", [HD, IN], f32, isOutput=False)
    Al_in = nc.declare_dram_parameter("Al", [HD, 4], f32, isOutput=False)
    Ar_in = nc.declare_dram_parameter("Ar", [HD, 4], f32, isOutput=False)
    h_out = nc.declare_dram_parameter("h_out", [NODES_PER_CORE, HD], f32,
                                      isOutput=True)
    elr_out = nc.declare_dram_parameter("elr_out", [NODES_PER_CORE, 8], f32,
                                        isOutput=True)

    with tile.TileContext(nc) as tc:
        with tc.tile_pool(name="cst", bufs=1) as cst, \
             tc.tile_pool(name="sb", bufs=3) as sb, \
             tc.tile_pool(name="ps", bufs=3, space="PSUM") as ps, \
             tc.tile_pool(name="psw", bufs=1, space="PSUM") as psw:

            # WLR = [W | Wl | Wr] where Wl = W @ Al, Wr = W @ Ar
            wt_sb = cst.tile([HD, IN], f32, tag="wt")
            nc.sync.dma_start(out=wt_sb[:], in_=WT_in[:])
            al_sb = cst.tile([HD, 4], f32, tag="al")
            nc.sync.dma_start(out=al_sb[:], in_=Al_in[:])
            ar_sb = cst.tile([HD, 4], f32, tag="ar")
            nc.sync.dma_start(out=ar_sb[:], in_=Ar_in[:])

            wlr = cst.tile([IN, 136], f32, tag="wlr")
            nc.sync.dma_start(out=wlr[:, 0:HD], in_=W_in[:])
            wl_ps = psw.tile([IN, 8], f32, tag="wlp")
            nc.tensor.matmul(out=wl_ps[:, 0:4], lhsT=wt_sb[:], rhs=al_sb[:],
                             start=True, stop=True)
            nc.tensor.matmul(out=wl_ps[:, 4:8], lhsT=wt_sb[:], rhs=ar_sb[:],
                             start=True, stop=True)
            nc.scalar.activation(out=wlr[:, 128:136], in_=wl_ps[:],
                                 func=mybir.ActivationFunctionType.Copy)

            # split-bf16 weights: PE fp32 is only ~2^-11 accurate, so run
            # hi/lo bf16 matmuls (exact products, fp32 accumulate)
            bf16 = mybir.dt.bfloat16
            wlr_hi = cst.tile([IN, 136], bf16, tag="wlrh")
            nc.scalar.activation(out=wlr_hi[:], in_=wlr[:],
                                 func=mybir.ActivationFunctionType.Copy)
            wlr_hi_f = cst.tile([IN, 136], f32, tag="wlrhf")
            nc.scalar.activation(out=wlr_hi_f[:], in_=wlr_hi[:],
                                 func=mybir.ActivationFunctionType.Copy)
            wlr_lo_f = cst.tile([IN, 136], f32, tag="wlrlf")
            nc.vector.tensor_tensor(out=wlr_lo_f[:], in0=wlr[:],
                                    in1=wlr_hi_f[:],
                                    op=mybir.AluOpType.subtract)
            wlr_lo = cst.tile([IN, 136], bf16, tag="wlrl")
            nc.scalar.activation(out=wlr_lo[:], in_=wlr_lo_f[:],
                                 func=mybir.ActivationFunctionType.Copy)

            CH = 7  # tiles per chunk; 98 = 14 chunks of 7
            n_chunks = NODES_PER_CORE // (P * CH)
            for c in range(n_chunks):
                ft = sb.tile([P, CH * P], f32, tag="ft")
                nc.sync.dma_start(
                    out=ft[:], in_=featT[:, c * CH * P:(c + 1) * CH * P])
                ft_hi = sb.tile([P, CH * P], bf16, tag="fth")
                nc.scalar.activation(out=ft_hi[:], in_=ft[:],
                                     func=mybir.ActivationFunctionType.Copy)
                ft_hi_f = sb.tile([P, CH * P], f32, tag="fthf")
                nc.scalar.activation(out=ft_hi_f[:], in_=ft_hi[:],
                                     func=mybir.ActivationFunctionType.Copy)
                ft_lo_f = sb.tile([P, CH * P], f32, tag="ftlf")
                nc.vector.tensor_tensor(out=ft_lo_f[:], in0=ft[:],
                                        in1=ft_hi_f[:],
                                        op=mybir.AluOpType.subtract)
                ft_lo = sb.tile([P, CH * P], bf16, tag="ftl")
                nc.vector.tensor_copy(ft_lo[:], ft_lo_f[:])
                hsb = sb.tile([P, CH * HD], f32, tag="hsb")
                esb = sb.tile([P, CH * 8], f32, tag="esb")
                for t in range(CH):
                    hp = ps.tile([P, 136], f32, tag="hp")
                    nc.tensor.matmul(out=hp[:],
                                     lhsT=ft_hi[:, t * P:(t + 1) * P],
                                     rhs=wlr_hi[:], start=True, stop=False)
                    nc.tensor.matmul(out=hp[:],
                                     lhsT=ft_hi[:, t * P:(t + 1) * P],
                                     rhs=wlr_lo[:], start=False, stop=False)
                    nc.tensor.matmul(out=hp[:],
                                     lhsT=ft_lo[:, t * P:(t + 1) * P],
                                     rhs=wlr_hi[:], start=False, stop=True)
                    nc.scalar.activation(
                        out=hsb[:, t * HD:(t + 1) * HD], in_=hp[:, 0:HD],
                        func=mybir.ActivationFunctionType.Copy)
                    nc.vector.tensor_copy(esb[:, t * 8:(t + 1) * 8],
                                          hp[:, 128:136])
                nc.sync.dma_start(
                    out=h_out[c * CH * P:(c + 1) * CH * P, :].rearrange(
                        "(k p) f -> p k f", p=P),
                    in_=hsb[:].rearrange("p (k f) -> p k f", k=CH))
                nc.sync.dma_start(
                    out=elr_out[c * CH * P:(c + 1) * CH * P, :].rearrange(
                        "(k p) f -> p k f", p=P),
                    in_=esb[:].rearrange("p (k f) -> p k f", k=CH))
    nc.compile()
    return nc


# ---------------- invocation 2: edge aggregation ----------------
def _build_inv2(T):
    """T = tiles per window (uniform across all cores/windows)."""
    nc = bacc.Bacc("TRN2", target_bir_lowering=False, debug=False,
                   num_devices=NCORES)
    hsrc = nc.declare_dram_parameter(
        "hsrc", [WIN_PER_CORE * P, T * HD], f32, isOutput=False)
    meta = nc.declare_dram_parameter(
        "meta", [WIN_PER_CORE * P, T * 10], f32, isOutput=False)
    bias_in = nc.declare_dram_parameter("bias", [1, HD], f32, isOutput=False)
    out_d = nc.declare_dram_parameter("out", [NODES_PER_CORE, D], f32,
                                      isOutput=True)

    KW = T * P  # free width of per-window chunk ops
    TA = min(7, T)  # tiles whose one-hot builds on ScalarE (Abs/Relu trick)

    with tile.TileContext(nc) as tc:
        with tc.tile_pool(name="cst", bufs=1) as cst, \
             tc.tile_pool(name="ld", bufs=3) as ld, \
             tc.tile_pool(name="wk", bufs=2) as wk, \
             tc.tile_pool(name="fl", bufs=2) as fl, \
             tc.tile_pool(name="ps", bufs=2, space="PSUM") as ps, \
             tc.tile_pool(name="psb", bufs=1, space="PSUM") as psb:

            # constants: iota row tile, bias_mean broadcast tile
            iota_row = cst.tile([P, P], f32, tag="iota")
            nc.gpsimd.iota(iota_row[:], pattern=[[1, P]], base=0,
                           channel_multiplier=0,
                           allow_small_or_imprecise_dtypes=True)
            bias_sb = cst.tile([1, HD], f32, tag="brow")
            nc.sync.dma_start(out=bias_sb[:], in_=bias_in[:])
            bias_m = cst.tile([1, D], f32, tag="bm")
            nc.vector.tensor_reduce(
                out=bias_m[:],
                in_=bias_sb[0:1, :].rearrange("p (h d) -> p d h", h=H),
                axis=mybir.AxisListType.X, op=mybir.AluOpType.add)
            nc.vector.tensor_scalar_mul(bias_m[:], bias_m[:], 1.0 / H)
            ones1 = cst.tile([1, P], f32, tag="ones")
            nc.vector.memset(ones1[:], 1.0)
            bias_ps = psb.tile([P, D], f32, tag="bps")
            nc.tensor.matmul(out=bias_ps[:], lhsT=ones1[:], rhs=bias_m[:],
                             start=True, stop=True)
            bias_bc = cst.tile([P, D], f32, tag="bbc")
            nc.vector.tensor_copy(bias_bc[:], bias_ps[:])

            for w in range(WIN_PER_CORE):
                base = w * KW
                # ---- loads (host provides [w*128+p, T*…] contiguous rows)
                hch = ld.tile([P, T * HD], f32, tag="hch")
                nc.sync.dma_start(
                    out=hch[:], in_=hsrc[w * P:(w + 1) * P, :])
                mch = ld.tile([P, T * 10], f32, tag="mch")
                nc.sync.dma_start(
                    out=mch[:], in_=meta[w * P:(w + 1) * P, :])
                mv = mch[:].rearrange("p (k f) -> p k f", k=T)

                # ---- one-hot B: first TA tiles on ACT, rest on DVE ----
                B = wk.tile([P, KW], f32, tag="B")
                for t in range(TA):
                    tmp = fl.tile([P, P], f32, tag="ohtmp")
                    nc.scalar.activation(
                        out=tmp[:], in_=iota_row[:],
                        func=mybir.ActivationFunctionType.Abs,
                        bias=mv[:, t, 9:10])
                    nc.scalar.activation(
                        out=B[:, t * P:(t + 1) * P], in_=tmp[:],
                        func=mybir.ActivationFunctionType.Relu,
                        scale=-1.0, bias=1.0)
                nc.vector.tensor_tensor(
                    out=B[:, TA * P:].rearrange("p (k v) -> p k v", k=T - TA),
                    in0=mv[:, TA:, 8:9].to_broadcast([P, T - TA, P]),
                    in1=iota_row[:].unsqueeze(1).to_broadcast(
                        [P, T - TA, P]),
                    op=mybir.AluOpType.is_equal)

                # ---- logits -> expE, msg ----
                msg = wk.tile([P, T * 132], f32, tag="msg")
                msgv = msg[:].rearrange("p (k f) -> p k f", k=T)
                lg = fl.tile([P, T * 4], f32, tag="lg")
                nc.vector.tensor_tensor(
                    out=lg[:].rearrange("p (k f) -> p k f", k=T),
                    in0=mv[:, :, 0:4], in1=mv[:, :, 4:8],
                    op=mybir.AluOpType.add)
                # exp(leaky(x)) = max(exp(x), exp(NEG*x)) (exp monotone)
                e1 = fl.tile([P, T * 4], f32, tag="e1")
                nc.scalar.activation(out=e1[:], in_=lg[:],
                                     func=mybir.ActivationFunctionType.Exp)
                e2 = fl.tile([P, T * 4], f32, tag="e2")
                nc.scalar.activation(out=e2[:], in_=lg[:], scale=NEG,
                                     func=mybir.ActivationFunctionType.Exp)
                nc.vector.tensor_tensor(
                    out=msgv[:, :, 128:132],
                    in0=e1[:].rearrange("p (k f) -> p k f", k=T),
                    in1=e2[:].rearrange("p (k f) -> p k f", k=T),
                    op=mybir.AluOpType.max)
                # msg[:, :, 0:128] = h * expE (broadcast over D)
                nc.vector.tensor_tensor(
                    out=msgv[:, :, 0:128].rearrange(
                        "p k (h d) -> p k h d", h=H),
                    in0=hch[:].rearrange("p (k h d) -> p k h d", k=T, h=H),
                    in1=msgv[:, :, 128:132].unsqueeze(3).to_broadcast(
                        [P, T, H, D]),
                    op=mybir.AluOpType.mult)

                # ---- accumulate [U | s] over tiles ----
                acc = ps.tile([P, 132], f32, tag="acc")
                for t in range(T):
                    nc.tensor.matmul(
                        out=acc[:],
                        lhsT=B[:, t * P:(t + 1) * P],
                        rhs=msg[:, t * 132:(t + 1) * 132],
                        start=(t == 0), stop=(t == T - 1))

                # ---- flush: out = mean_h(U/s) + bias_mean (ACT-heavy) ----
                r4 = fl.tile([P, 4], f32, tag="r4")
                nc.vector.tensor_scalar_max(r4[:], acc[:, 128:132], 1e-30)
                nc.vector.reciprocal(r4[:], r4[:])
                nc.vector.tensor_scalar_mul(r4[:], r4[:], 1.0 / H)
                un = fl.tile([P, HD], f32, tag="un")
                for hh in range(H):
                    nc.scalar.activation(
                        out=un[:, hh * D:(hh + 1) * D],
                        in_=acc[:, hh * D:(hh + 1) * D],
                        func=mybir.ActivationFunctionType.Copy,
                        scale=r4[:, hh:hh + 1])
                red = fl.tile([P, D], f32, tag="red")
                nc.vector.tensor_reduce(
                    out=red[:],
                    in_=un[:].rearrange("p (h d) -> p d h", h=H),
                    axis=mybir.AxisListType.X, op=mybir.AluOpType.add)
                outt = fl.tile([P, D], f32, tag="outt")
                nc.vector.tensor_tensor(out=outt[:], in0=red[:],
                                        in1=bias_bc[:],
                                        op=mybir.AluOpType.add)
                nc.sync.dma_start(out=out_d[w * P:(w + 1) * P, :],
                                  in_=outt[:])
    nc.compile()
    return nc


_INV1 = None
_INV2 = {}
LAST_EXEC_NS = None
LAST_EXEC_NS1 = None
LAST_EXEC_NS2 = None
import os
_TRACE = bool(os.environ.get("GAT_TRACE"))


def kernel(feat, W, attn_l, attn_r, bias, src, dst):
    global _INV1, LAST_EXEC_NS, LAST_EXEC_NS1, LAST_EXEC_NS2
    feat = np.asarray(feat, dtype=np.float32)
    W = np.asarray(W, dtype=np.float32)
    attn_l = np.asarray(attn_l, dtype=np.float32)
    attn_r = np.asarray(attn_r, dtype=np.float32)
    bias = np.asarray(bias, dtype=np.float32)
    src = np.asarray(src, dtype=np.int32)
    dst = np.asarray(dst, dtype=np.int32)

    # ---------------- host: layout-only prep ----------------
    featT = np.zeros((IN, N_PAD), dtype=np.float32)
    featT[:, :N] = np.ascontiguousarray(feat.T)
    WT = np.ascontiguousarray(W.T)
    Al = np.zeros((HD, H), dtype=np.float32)
    Ar = np.zeros((HD, H), dtype=np.float32)
    for h in range(H):
        Al[h * D:(h + 1) * D, h] = attn_l[h]
        Ar[h * D:(h + 1) * D, h] = attn_r[h]

    # ---------------- inv-1: node tables ----------------
    if _INV1 is None:
        _INV1 = _build_inv1()
    in1 = []
    for c in range(NCORES):
        sl = slice(c * NODES_PER_CORE, (c + 1) * NODES_PER_CORE)
        in1.append({"featT": np.ascontiguousarray(featT[:, sl]),
                    "W": W, "WT": WT, "Al": Al, "Ar": Ar})
    res1 = run_bass_kernel_spmd(_INV1, in1, core_ids=list(range(NCORES)),
                                trace=_TRACE)
    LAST_EXEC_NS1 = res1.exec_time_ns
    h_full = np.concatenate([r["h_out"] for r in res1.results], axis=0)
    elr_full = np.concatenate([r["elr_out"] for r in res1.results], axis=0)

    # ---------------- host: edge slotting (index ops only) ----------------
    # Degree-balanced node->slot assignment: snake-assign nodes (sorted by
    # in-degree desc) across windows so per-window edge counts equalize.
    n_win_tot = NCORES * WIN_PER_CORE
    deg = np.bincount(dst, minlength=N)
    order = np.argsort(-deg, kind="stable")          # nodes, heavy first
    wseq = np.arange(N, dtype=np.int64) % (2 * n_win_tot)
    wseq = np.where(wseq < n_win_tot, wseq, 2 * n_win_tot - 1 - wseq)
    posc = np.zeros(n_win_tot, dtype=np.int64)
    # position of node within its window = running count per window
    posn = np.zeros(N, dtype=np.int64)
    # vectorized running count: for snake pattern, node i is the
    # (i // (2*n_win_tot))*2 + {0,1}-th member of its window... simpler:
    # each full snake pass hits every window exactly twice.
    pass_idx = np.arange(N, dtype=np.int64) // (2 * n_win_tot)
    within = np.arange(N, dtype=np.int64) % (2 * n_win_tot)
    posn = 2 * pass_idx + (within >= n_win_tot)
    node_slot = np.empty(N, dtype=np.int64)
    node_slot[order] = wseq * P + posn
    assert posn.max() < P

    slot_of_dst = node_slot[dst]
    perm = np.argsort(slot_of_dst, kind="stable")
    srcp = src[perm]
    dslot = slot_of_dst[perm]
    win = dslot >> 7
    counts = np.bincount(win, minlength=n_win_tot)
    T = max(1, int(math.ceil(counts.max() / P)))
    win_start = np.zeros(n_win_tot + 1, dtype=np.int64)
    np.cumsum(counts, out=win_start[1:])
    offs = np.arange(E, dtype=np.int64) - win_start[win]
    slot = win * (T * P) + offs

    S_tot = n_win_tot * T * P
    slot_src = np.zeros(S_tot, dtype=np.int64)
    slot_dstg = np.zeros(S_tot, dtype=np.int64)
    slot_dloc = np.full(S_tot, 999.0, dtype=np.float32)
    valid = np.zeros(S_tot, dtype=bool)
    slot_src[slot] = srcp
    slot_dstg[slot] = dst[perm]
    slot_dloc[slot] = (dslot & 127).astype(np.float32)
    valid[slot] = True

    hsrc_all = h_full[slot_src]              # [S_tot, 128]
    hsrc_all[~valid] = 0.0
    el_s = elr_full[slot_src][:, 0:4]
    er_s = elr_full[slot_dstg][:, 4:8]
    el_s[~valid] = 0.0
    er_s[~valid] = 0.0
    meta_all = np.concatenate(
        [el_s, er_s, slot_dloc[:, None], -slot_dloc[:, None]],
        axis=1).astype(np.float32)

    # per-partition-contiguous layouts: row (w*128+p) = concat over t
    hsrc_lay = np.ascontiguousarray(
        hsrc_all.reshape(n_win_tot, T, P, HD).transpose(0, 2, 1, 3)
        .reshape(n_win_tot * P, T * HD))
    meta_lay = np.ascontiguousarray(
        meta_all.reshape(n_win_tot, T, P, 10).transpose(0, 2, 1, 3)
        .reshape(n_win_tot * P, T * 10))

    # ---------------- inv-2: edge aggregation ----------------
    if T not in _INV2:
        _INV2[T] = _build_inv2(T)
    R_core = WIN_PER_CORE * P
    in2 = []
    for c in range(NCORES):
        sl = slice(c * R_core, (c + 1) * R_core)
        in2.append({"hsrc": hsrc_lay[sl],
                    "meta": meta_lay[sl],
                    "bias": bias.reshape(1, HD)})
    res2 = run_bass_kernel_spmd(_INV2[T], in2, core_ids=list(range(NCORES)),
                                trace=_TRACE)
    LAST_EXEC_NS2 = res2.exec_time_ns
    if LAST_EXEC_NS1 is not None and LAST_EXEC_NS2 is not None:
        LAST_EXEC_NS = LAST_EXEC_NS1 + LAST_EXEC_NS2
    dev_out = np.concatenate([r["out"] for r in res2.results], axis=0)
    return np.ascontiguousarray(dev_out[node_slot])



# revision 2
# speedup vs baseline: 1.0485x; 1.0485x over previous
"""AggrGATConv Trainium2 kernel v2: identity-layout edge aggregation.

Design:
  inv-1 (device, node-sharded): h = feat @ W (split-bf16 exact), el/er tables.
  host (index-only): sort nodes by in-degree desc; octet k = 1024 consecutive
    sorted nodes -> 8 windows of 128 (one per core, snaked), padded to the
    octet max degree T_k. Row p of window k IS dst node -> the scatter matrix
    is the identity (no one-hot build, no dloc). Padding slots get el=-1e4
    so exp()=0. Host gathers h32[src]/el[src] rows (data movement only).
  inv-2 (device, edge-sharded, single pass): per window:
    lg = el + er_bcast; expE = max(exp(lg), exp(0.2 lg));
    s = reduce_t(expE); r = 1/max(s,eps);
    wmsg = hch * expE (fp32); U = sum_t wmsg_t via PE matmul with
    lhsT = 0.25*I (PSUM accumulate);
    out = sum_h(U_h * r_h) + bias_mean  (0.25 head-mean folded into I).
"""
import sys
import types
import contextlib
import ctypes
import os

import numpy as np

import concourse.bacc as bacc
import concourse.tile as tile
import concourse.mybir as mybir
from concourse.bass_utils import run_bass_kernel_spmd

# ---------------- constants (hardcoded per problem spec) ----------------
N = 100000
E = 1600000
IN = 128
H, D = 4, 32
HD = H * D  # 128
NEG = 0.2
NCORES = 8
P = 128
K_WIN = 98                       # octets: 100352 / 1024
N_PAD = NCORES * K_WIN * P       # 100352
NODES_PER_CORE = K_WIN * P       # 12544
PAD_LOGIT = -100.0  # exp(-100)=0, exp(-20)=2e-9; stays in ACT table range

f32 = mybir.dt.float32
i32 = mybir.dt.int32


def _install_ntff_shim():
    """antenv.axon_hooks is absent in this image; provide the ctypes hook so
    trace=True works (used by test harness; harmless otherwise)."""
    if "antenv.axon_hooks" in sys.modules:
        return
    try:
        lib = ctypes.CDLL("/opt/axon/libaxon_pjrt.so")
        if not hasattr(lib, "axon_start_nrt_profile"):
            raise OSError("no symbol")
        lib.axon_start_nrt_profile.argtypes = [
            ctypes.POINTER(ctypes.c_int64), ctypes.c_size_t]
        lib.axon_start_nrt_profile.restype = ctypes.c_int64
        lib.axon_stop_nrt_profile.argtypes = [ctypes.c_char_p]
        lib.axon_stop_nrt_profile.restype = ctypes.c_int64

        @contextlib.contextmanager
        def _hook(output_dir, device_ids):
            import jax
            jax.devices()
            if device_ids:
                ids = (ctypes.c_int64 * len(device_ids))(*device_ids)
                rc = lib.axon_start_nrt_profile(ids, len(device_ids))
            else:
                rc = lib.axon_start_nrt_profile(None, 0)
            if rc != 0:
                raise RuntimeError(f"axon_start_nrt_profile rc={rc}")
            try:
                yield
            finally:
                n = lib.axon_stop_nrt_profile(str(output_dir).encode())
                print(f"profile: {n} file(s) -> {output_dir}", file=sys.stderr)

        hook = _hook
    except OSError:
        hook = None
    mod = types.ModuleType("antenv.axon_hooks")
    mod.get_axon_ntff_profile_hook = lambda: hook
    mod.set_axon_ntff_profile_hook = lambda h: None
    sys.modules["antenv.axon_hooks"] = mod


_install_ntff_shim()


# ---------------- invocation 1: node tables ----------------
def _build_inv1():
    nc = bacc.Bacc("TRN2", target_bir_lowering=False, debug=False,
                   num_devices=NCORES)
    featT = nc.declare_dram_parameter("featT", [P, NODES_PER_CORE], f32,
                                      isOutput=False)
    W_in = nc.declare_dram_parameter("W", [IN, HD], f32, isOutput=False)
    WT_in = nc.declare_dram_parameter("WT", [HD, IN], f32, isOutput=False)
    Al_in = nc.declare_dram_parameter("Al", [HD, 4], f32, isOutput=False)
    Ar_in = nc.declare_dram_parameter("Ar", [HD, 4], f32, isOutput=False)
    # partition-major outputs: row p holds all its tiles contiguously
    # (fat DMA descriptors); host reshapes back to node-major for free.
    h_out = nc.declare_dram_parameter("h_out", [P, K_WIN * HD], f32,
                                      isOutput=True)
    elr_out = nc.declare_dram_parameter("elr_out", [P, K_WIN * 8], f32,
                                        isOutput=True)

    with tile.TileContext(nc) as tc:
        with tc.tile_pool(name="cst", bufs=1) as cst, \
             tc.tile_pool(name="sb", bufs=3) as sb, \
             tc.tile_pool(name="ps", bufs=3, space="PSUM") as ps, \
             tc.tile_pool(name="psw", bufs=1, space="PSUM") as psw:

            # WLR = [W | Wl | Wr] where Wl = W @ Al, Wr = W @ Ar
            wt_sb = cst.tile([HD, IN], f32, tag="wt")
            nc.sync.dma_start(out=wt_sb[:], in_=WT_in[:])
            al_sb = cst.tile([HD, 4], f32, tag="al")
            nc.sync.dma_start(out=al_sb[:], in_=Al_in[:])
            ar_sb = cst.tile([HD, 4], f32, tag="ar")
            nc.sync.dma_start(out=ar_sb[:], in_=Ar_in[:])

            wlr = cst.tile([IN, 136], f32, tag="wlr")
            nc.sync.dma_start(out=wlr[:, 0:HD], in_=W_in[:])
            wl_ps = psw.tile([IN, 8], f32, tag="wlp")
            nc.tensor.matmul(out=wl_ps[:, 0:4], lhsT=wt_sb[:], rhs=al_sb[:],
                             start=True, stop=True)
            nc.tensor.matmul(out=wl_ps[:, 4:8], lhsT=wt_sb[:], rhs=ar_sb[:],
                             start=True, stop=True)
            nc.scalar.activation(out=wlr[:, 128:136], in_=wl_ps[:],
                                 func=mybir.ActivationFunctionType.Copy)

            CH = 14  # tiles per chunk; 98 = 7 chunks of 14
            n_chunks = NODES_PER_CORE // (P * CH)
            for c in range(n_chunks):
                ft = sb.tile([P, CH * P], f32, tag="ft")
                nc.sync.dma_start(
                    out=ft[:], in_=featT[:, c * CH * P:(c + 1) * CH * P])
                hsb = sb.tile([P, CH * HD], f32, tag="hsb")
                esb = sb.tile([P, CH * 8], f32, tag="esb")
                for t in range(CH):
                    hp = ps.tile([P, 136], f32, tag="hp")
                    # fp32 matmul mode is exact (2 half-speed passes)
                    nc.tensor.matmul(out=hp[:],
                                     lhsT=ft[:, t * P:(t + 1) * P],
                                     rhs=wlr[:], start=True, stop=True)
                    if t % 2 == 0:
                        nc.scalar.activation(
                            out=hsb[:, t * HD:(t + 1) * HD], in_=hp[:, 0:HD],
                            func=mybir.ActivationFunctionType.Copy)
                    else:
                        nc.vector.tensor_copy(hsb[:, t * HD:(t + 1) * HD],
                                              hp[:, 0:HD])
                    nc.vector.tensor_copy(esb[:, t * 8:(t + 1) * 8],
                                          hp[:, 128:136])
                nc.gpsimd.dma_start(
                    out=h_out[:, c * CH * HD:(c + 1) * CH * HD],
                    in_=hsb[:])
                nc.gpsimd.dma_start(
                    out=elr_out[:, c * CH * 8:(c + 1) * CH * 8],
                    in_=esb[:])
    nc.compile()
    return nc


# ---------------- invocation 2: identity-layout edge aggregation ----------
def _build_inv2(Ts, mm_pairs=True, use_gpsimd=True):
    """Ts = per-window tile counts (desc), uniform across cores."""
    Ts = list(Ts)
    CAP = sum(Ts) * P           # hch cols
    CAP4 = sum(Ts) * 4          # el cols
    TMAX = max(max(Ts), 1)
    nc = bacc.Bacc("TRN2", target_bir_lowering=False, debug=False,
                   num_devices=NCORES)
    hsrc = nc.declare_dram_parameter("hsrc", [P, CAP], f32, isOutput=False)
    el_d = nc.declare_dram_parameter("el", [P, CAP4], f32, isOutput=False)
    er_d = nc.declare_dram_parameter("er", [P, K_WIN * 4], f32,
                                     isOutput=False)
    ident_d = nc.declare_dram_parameter("ident", [P, P], f32, isOutput=False)
    bias_in = nc.declare_dram_parameter("bias", [1, HD], f32, isOutput=False)
    out_d = nc.declare_dram_parameter("out", [P, K_WIN * D], f32,
                                      isOutput=True)

    Exp = mybir.ActivationFunctionType.Exp
    Copy = mybir.ActivationFunctionType.Copy
    Add = mybir.AluOpType.add
    Mult = mybir.AluOpType.mult
    Max = mybir.AluOpType.max

    with tile.TileContext(nc) as tc:
        with tc.tile_pool(name="cst", bufs=1) as cst, \
             tc.tile_pool(name="ld", bufs=3) as ld, \
             tc.tile_pool(name="wk", bufs=3) as wk, \
             tc.tile_pool(name="fl", bufs=3) as fl, \
             tc.tile_pool(name="ps", bufs=4, space="PSUM") as ps, \
             tc.tile_pool(name="psb", bufs=1, space="PSUM") as psb:

            # constants
            ident = cst.tile([P, P], f32, tag="ident")
            nc.sync.dma_start(out=ident[:], in_=ident_d[:])
            er_all = cst.tile([P, K_WIN * 4], f32, tag="erall")
            nc.sync.dma_start(out=er_all[:], in_=er_d[:])
            bias_sb = cst.tile([1, HD], f32, tag="brow")
            nc.sync.dma_start(out=bias_sb[:], in_=bias_in[:])
            bias_m = cst.tile([1, D], f32, tag="bm")
            nc.vector.tensor_reduce(
                out=bias_m[:],
                in_=bias_sb[0:1, :].rearrange("p (h d) -> p d h", h=H),
                axis=mybir.AxisListType.X, op=Add)
            nc.vector.tensor_scalar_mul(bias_m[:], bias_m[:], 1.0 / H)
            ones1 = cst.tile([1, P], f32, tag="ones")
            nc.vector.memset(ones1[:], 1.0)
            bias_ps = psb.tile([P, D], f32, tag="bps")
            nc.tensor.matmul(out=bias_ps[:], lhsT=ones1[:], rhs=bias_m[:],
                             start=True, stop=True)
            bias_bc = cst.tile([P, D], f32, tag="bbc")
            nc.vector.tensor_copy(bias_bc[:], bias_ps[:])

            off = 0  # running tile offset
            for k, T in enumerate(Ts):
                if T == 0:
                    # isolated octet: out = bias_mean
                    nc.sync.dma_start(
                        out=out_d[:, k * D:(k + 1) * D], in_=bias_bc[:])
                    continue
                KW = T * P
                hch = ld.tile([P, TMAX * P], f32, tag="hch")
                KW2 = (T // 2) * P
                if KW2 > 0:
                    nc.sync.dma_start(
                        out=hch[:, 0:KW2],
                        in_=hsrc[:, off * P:off * P + KW2])
                    nc.sync.dma_start(
                        out=hch[:, KW2:KW],
                        in_=hsrc[:, off * P + KW2:off * P + KW])
                else:
                    nc.sync.dma_start(
                        out=hch[:, 0:KW],
                        in_=hsrc[:, off * P:off * P + KW])
                elch = ld.tile([P, TMAX * 4], f32, tag="elch")
                nc.gpsimd.dma_start(
                    out=elch[:, 0:T * 4],
                    in_=el_d[:, off * 4:off * 4 + T * 4])

                # logits -> expE
                eng_a = nc.gpsimd if use_gpsimd else nc.vector
                lg = fl.tile([P, TMAX * 4], f32, tag="lg")
                nc.vector.tensor_tensor(
                    out=lg[:, 0:T * 4].rearrange("p (t h) -> p t h", t=T),
                    in0=elch[:, 0:T * 4].rearrange("p (t h) -> p t h", t=T),
                    in1=er_all[:, k * 4:(k + 1) * 4].unsqueeze(1)
                        .to_broadcast([P, T, 4]),
                    op=Add)
                e1 = fl.tile([P, TMAX * 4], f32, tag="e1")
                nc.scalar.activation(out=e1[:, 0:T * 4], in_=lg[:, 0:T * 4],
                                     func=Exp)
                e2 = fl.tile([P, TMAX * 4], f32, tag="e2")
                nc.scalar.activation(out=e2[:, 0:T * 4], in_=lg[:, 0:T * 4],
                                     scale=NEG, func=Exp)
                expE = fl.tile([P, TMAX * 4], f32, tag="expE")
                eng_a.tensor_tensor(out=expE[:, 0:T * 4],
                                    in0=e1[:, 0:T * 4],
                                    in1=e2[:, 0:T * 4], op=Max)

                # s, r per head
                s4 = fl.tile([P, 4], f32, tag="s4")
                nc.vector.tensor_reduce(
                    out=s4[:],
                    in_=expE[:, 0:T * 4].rearrange("p (t h) -> p h t", t=T),
                    axis=mybir.AxisListType.X, op=Add)
                r4 = fl.tile([P, 4], f32, tag="r4")
                nc.vector.tensor_scalar_max(r4[:], s4[:], 1e-30)
                nc.vector.reciprocal(r4[:], r4[:])

                # wmsg = hch * expE (broadcast over D), split in two halves
                # so PE matmuls can start after the first half completes
                wmsg = wk.tile([P, TMAX * P], f32, tag="wmsg")
                Ta = max(1, T // 2)
                for (ta, tb) in ((0, Ta), (Ta, T)):
                    tn = tb - ta
                    if tn <= 0:
                        continue
                    nc.vector.tensor_tensor(
                        out=wmsg[:, ta * P:tb * P].rearrange(
                            "p (t h d) -> p t h d", t=tn, h=H),
                        in0=hch[:, ta * P:tb * P].rearrange(
                            "p (t h d) -> p t h d", t=tn, h=H),
                        in1=expE[:, ta * 4:tb * 4].rearrange(
                            "p (t h) -> p t h", t=tn).unsqueeze(3)
                            .to_broadcast([P, tn, H, D]),
                        op=Mult)

                # U = sum_t wmsg_t  (identity matmul, PSUM accumulate;
                # 0.25 head-mean folded into ident)
                f32r = mybir.dt.float32r
                # balance the tile-sum between PE (fp32 matmul, ~244ns/tile)
                # and DVE (fp32 adds, ~230ns/tile): DVE takes the last ndve
                # tiles, PE the rest; combine with one add.
                if mm_pairs:
                    smalls = 605 if use_gpsimd else 1350
                    best, ndve = None, 0
                    for n in range(0, T):
                        dve_t = 2220 + smalls + (230 if n else 0) + 230 * n
                        pe_t = 244 * (T - n)
                        m = max(dve_t, pe_t)
                        if best is None or m < best:
                            best, ndve = m, n
                else:
                    ndve = 0
                npe = T - ndve
                acc = ps.tile([P, HD], f32, tag="acc")
                for t in range(npe):
                    nc.tensor.matmul(
                        out=acc[:], lhsT=ident[:],
                        rhs=wmsg[:, t * P:(t + 1) * P],
                        start=(t == 0), stop=(t == npe - 1))
                if ndve > 0:
                    if ndve == 1:
                        # combine directly below
                        dsrc = wmsg[:, npe * P:(npe + 1) * P]
                    else:
                        udve = fl.tile([P, HD], f32, tag="udve")
                        nc.vector.tensor_tensor(
                            out=udve[:], in0=wmsg[:, npe * P:(npe + 1) * P],
                            in1=wmsg[:, (npe + 1) * P:(npe + 2) * P], op=Add)
                        for t in range(npe + 2, T):
                            nc.vector.tensor_tensor(
                                out=udve[:], in0=udve[:],
                                in1=wmsg[:, t * P:(t + 1) * P], op=Add)
                        dsrc = udve[:]
                    u128 = fl.tile([P, HD], f32, tag="u128")
                    # note: identity carries the 0.25 head-mean fold, so
                    # DVE-summed tiles must also be scaled by 0.25 -> do it
                    # in the combine via scalar_tensor_tensor:
                    # u128 = (dsrc * 0.25) + acc
                    nc.vector.scalar_tensor_tensor(
                        out=u128[:], in0=dsrc, scalar=0.25, in1=acc[:],
                        op0=Mult, op1=Add)
                    usrc = u128
                else:
                    usrc = acc

                # flush: out = sum_h (U_h * r_h) + bias_mean
                un = fl.tile([P, HD], f32, tag="un")
                for hh in range(H):
                    nc.scalar.activation(
                        out=un[:, hh * D:(hh + 1) * D],
                        in_=usrc[:, hh * D:(hh + 1) * D],
                        func=Copy, scale=r4[:, hh:hh + 1])
                red = fl.tile([P, D], f32, tag="red")
                nc.vector.tensor_reduce(
                    out=red[:],
                    in_=un[:].rearrange("p (h d) -> p d h", h=H),
                    axis=mybir.AxisListType.X, op=Add)
                outt = fl.tile([P, D], f32, tag="outt")
                eng_a.tensor_tensor(out=outt[:], in0=red[:],
                                    in1=bias_bc[:], op=Add)
                nc.gpsimd.dma_start(out=out_d[:, k * D:(k + 1) * D],
                                    in_=outt[:])
                off += T
    nc.compile()
    return nc


_INV1 = None
_INV2 = {}
LAST_EXEC_NS = None
LAST_EXEC_NS1 = None
LAST_EXEC_NS2 = None
_TRACE = bool(os.environ.get("GAT_TRACE"))


def kernel(feat, W, attn_l, attn_r, bias, src, dst):
    global _INV1, LAST_EXEC_NS, LAST_EXEC_NS1, LAST_EXEC_NS2
    feat = np.asarray(feat, dtype=np.float32)
    W = np.asarray(W, dtype=np.float32)
    attn_l = np.asarray(attn_l, dtype=np.float32)
    attn_r = np.asarray(attn_r, dtype=np.float32)
    bias = np.asarray(bias, dtype=np.float32)
    src = np.asarray(src, dtype=np.int32)
    dst = np.asarray(dst, dtype=np.int32)

    # ---------------- host: layout-only prep ----------------
    featT = np.zeros((IN, N_PAD), dtype=np.float32)
    featT[:, :N] = np.ascontiguousarray(feat.T)
    WT = np.ascontiguousarray(W.T)
    Al = np.zeros((HD, H), dtype=np.float32)
    Ar = np.zeros((HD, H), dtype=np.float32)
    for h in range(H):
        Al[h * D:(h + 1) * D, h] = attn_l[h]
        Ar[h * D:(h + 1) * D, h] = attn_r[h]

    # ---------------- inv-1: node tables ----------------
    if _INV1 is None:
        _INV1 = _build_inv1()
    in1 = []
    for c in range(NCORES):
        sl = slice(c * NODES_PER_CORE, (c + 1) * NODES_PER_CORE)
        in1.append({"featT": np.ascontiguousarray(featT[:, sl]),
                    "W": W, "WT": WT, "Al": Al, "Ar": Ar})
    res1 = run_bass_kernel_spmd(_INV1, in1, core_ids=list(range(NCORES)),
                                trace=_TRACE)
    LAST_EXEC_NS1 = res1.exec_time_ns
    h_full = np.concatenate(
        [r["h_out"].reshape(P, K_WIN, HD).transpose(1, 0, 2)
         .reshape(NODES_PER_CORE, HD) for r in res1.results], axis=0)
    elr_full = np.concatenate(
        [r["elr_out"].reshape(P, K_WIN, 8).transpose(1, 0, 2)
         .reshape(NODES_PER_CORE, 8) for r in res1.results], axis=0)

    # ---------------- host: identity-layout slotting (index ops only) -----
    deg = np.bincount(dst, minlength=N_PAD).astype(np.int64)
    order = np.argsort(-deg, kind="stable")
    rank = np.empty(N_PAD, dtype=np.int64)
    rank[order] = np.arange(N_PAD)
    k_of = rank >> 10                  # octet index (node -> window)
    within = rank & 1023
    c_of = within >> 7
    c_of = np.where(k_of & 1 == 1, NCORES - 1 - c_of, c_of)  # snake
    p_of = within & 127

    Ts = deg[order[::1024]]            # max deg per octet (sorted desc)
    Ts = np.maximum(Ts, 0)
    key = tuple(int(t) for t in Ts)
    tile_off = np.zeros(K_WIN + 1, dtype=np.int64)
    np.cumsum(Ts, out=tile_off[1:])
    CAP = int(tile_off[-1]) * P

    # per-edge slot: t = running count within dst node
    perm = np.argsort(dst, kind="stable")
    dstp = dst[perm]
    srcp = src[perm]
    estart = np.zeros(N_PAD + 1, dtype=np.int64)
    np.cumsum(np.bincount(dstp, minlength=N_PAD), out=estart[1:])
    t_of = np.arange(E, dtype=np.int64) - estart[dstp]
    ce = c_of[dstp]
    pe = p_of[dstp]
    tile_e = tile_off[k_of[dstp]] + t_of

    # gathers (pure data movement)
    n_tiles = int(tile_off[-1])
    hsrc_lay = np.zeros((NCORES, P, n_tiles, HD), dtype=np.float32)
    hsrc_lay[ce, pe, tile_e, :] = h_full[srcp]
    el_lay = np.full((NCORES, P, n_tiles, 4), PAD_LOGIT, dtype=np.float32)
    el_lay[ce, pe, tile_e, :] = elr_full[srcp][:, 0:4]
    er_lay = np.zeros((NCORES, P, K_WIN, 4), dtype=np.float32)
    nodes = np.arange(N_PAD)
    er_lay[c_of, p_of, k_of, :] = elr_full[nodes][:, 4:8]
    ident = (0.25 * np.eye(P)).astype(np.float32)

    # ---------------- inv-2: edge aggregation ----------------
    if key not in _INV2:
        _INV2[key] = _build_inv2(key, mm_pairs=True, use_gpsimd=False)
    in2 = []
    for c in range(NCORES):
        in2.append({"hsrc": hsrc_lay[c].reshape(P, CAP),
                    "el": el_lay[c].reshape(P, n_tiles * 4),
                    "er": er_lay[c].reshape(P, K_WIN * 4),
                    "ident": ident,
                    "bias": bias.reshape(1, HD)})
    res2 = run_bass_kernel_spmd(_INV2[key], in2, core_ids=list(range(NCORES)),
                                trace=_TRACE)
    LAST_EXEC_NS2 = res2.exec_time_ns
    if LAST_EXEC_NS1 is not None and LAST_EXEC_NS2 is not None:
        LAST_EXEC_NS = LAST_EXEC_NS1 + LAST_EXEC_NS2
    out_full = np.zeros((N_PAD, D), dtype=np.float32)
    res_arr = np.stack([r["out"].reshape(P, K_WIN, D)
                        for r in res2.results])  # [c, p, k, d]
    out_full[nodes] = res_arr[c_of, p_of, k_of, :]
    return np.ascontiguousarray(out_full[:N])
